# revision 8
# baseline (speedup 1.0000x reference)
"""Trainium2 kernel for nn_CNNEncoder: embed(1000,3) -> 4x conv1d(stride3) -> relu -> 50x50 linear.

Math: the four stride-3 convs + concat are one linear map C [50, 60] over the
flattened embedding signal e = emb[src].reshape(B, 60). The conv windows never
touch signal positions 50..59 (max live index is 49), so C[:, 50:] == 0 and the
shipped signal is trimmed to 50 live positions per row:
    out = relu(e_live @ C_live.T + cb) @ Wp.T + bp

Device layout (pure data parallel over 8 cores, 65536 rows/core):
  - features on partitions, rows on the free dim (PE contracts over partitions)
  - two 32768-row chunks packed block-diagonally: stage-1 lhsT is [101, 101]
    (50 live signal partitions per chunk + shared ones-row for the bias; col
    100 forwards the ones-row so stage 2 gets its bias row for free), stage-2
    lhsT is [101, 100].
  - column quarters of 8192: all 16 stage-1 matmuls (one stationary), relu
    (split ACT/DVE) into an SBUF h buffer, then all 16 stage-2 matmuls (one
    stationary), PSUM->SBUF fp16 cast (split DVE/ACT). Grouping the
    stationaries lets the PE's background weight buffer hide LDWEIGHTS;
    1024-col drains amortize per-op engine overhead.
  - input signal shipped as fp8 e3m4 (1 byte; ~1.2% rel err, well under the
    2e-2 gate) and consumed directly by the PE: TRN2 matmul supports a mixed
    fp16-stationary x fp8e3-moving matmul exactly (hw-verified).
  - ALL bulk DMA via SWDGE (gpsimd.dma_start): software descriptors spray
    evenly across all 16 SDMA engines (HWDGE concentrates on few engines).
    Issue order interleaves input prefetch BEFORE output stores so a store
    waiting on casts can never head-of-line block the next quarter's input.

Host side does only data movement: the embedding gather (index lookup, no
arithmetic) and transposes for the on-device layout. All FLOPs run on device.
"""

import os
import numpy as np
import ml_dtypes

try:
    import concourse.bass as bass
except ImportError:  # grading env may not have concourse on sys.path
    import sys

    sys.path.insert(0, "/opt/trn_rl_repo")
    import concourse.bass as bass

import concourse.mybir as mybir
import concourse.tile as tile
from concourse import bacc
from concourse.bass import ds, ts
from concourse import bass_utils
from concourse.bass_utils import run_bass_kernel_spmd

# spread HWDGE DMAs across many SDMA engines (default pins them to one)
_orig_run_command = bass_utils.run_command

_FLAG = "--min-num-dma-engines-for-dge=16"
_flag_ok = None


def _walrus_supports_flag(walrus):
    global _flag_ok
    if _flag_ok is None:
        try:
            import subprocess

            out = subprocess.run(
                [walrus, "--help"], capture_output=True, text=True, timeout=120
            )
            _flag_ok = "--min-num-dma-engines-for-dge" in (out.stdout + out.stderr)
        except Exception:
            _flag_ok = False
    return _flag_ok


def _patched_run_command(argv, **kwargs):
    if (
        argv
        and "walrus_driver" in str(argv[0])
        and "--pass" in argv
        and _walrus_supports_flag(str(argv[0]))
    ):
        argv = list(argv) + [_FLAG]
    return _orig_run_command(argv, **kwargs)


bass_utils.run_command = _patched_run_command


B = 524288
SEQ = 20
EMB = 3
L = SEQ * EMB  # 60 (only 0..49 live)
LIVE = 50
F = 50
NCORES = 8
RPC = B // NCORES  # 65536 rows per core
HALF = RPC // 2  # 32768 rows per packed chunk
NT = HALF  # free dim of the per-core device tensors

KP1 = 2 * LIVE + 1  # 101: [chunkA 50 | chunkB 50 | ones]
MP1 = 2 * F + 1  # 101: [chunkA 50 | chunkB 50 | ones passthrough]
KP2 = MP1  # 101
MP2 = 2 * F  # 100

QUART = 8192
DMA_N = 4096
SUB = 512
PS_N = 1024  # psum tile cols (2 banks)

F32 = mybir.dt.float32
F16 = mybir.dt.float16
F8E3 = mybir.dt.float8e3

CONV_SPECS = [(10, 14), (12, 13), (13, 12), (15, 11)]  # (pad, n_out)

LAST_RESULTS = None  # BassKernelResults of the most recent run (for profiling)

_NC_CACHE = {}


def _build_C(w1, b1, w2, b2, w3, b3, w4, b4):
    C = np.zeros((F, L), np.float64)
    cb = np.zeros(F, np.float64)
    f = 0
    for (w, b), (pad, nout) in zip(
        [(w1, b1), (w2, b2), (w3, b3), (w4, b4)], CONV_SPECS
    ):
        wk = np.asarray(w, np.float64)[0, 0]
        K = wk.shape[0]
        for j in range(nout):
            for k in range(K):
                i = 3 * j + k - pad
                if 0 <= i < L:
                    C[f, i] += wk[k]
            cb[f] = np.asarray(b, np.float64)[0]
            f += 1
    assert np.all(C[:, LIVE:] == 0.0)
    return C[:, :LIVE].astype(np.float32), cb.astype(np.float32)


def _stage1(nc, q, xts, h, w1t, ps1):
    # xts: list of (tile, base_col_in_quarter, ncols)
    relu = mybir.ActivationFunctionType.Relu
    for j in range(QUART // PS_N):
        p = ps1.tile([MP1, PS_N], F32, tag="p1", name=f"p1_{q}_{j}")
        col = j * PS_N
        xt, base, ncols = next(
            (t, b, n) for (t, b, n) in xts if b <= col < b + n
        )
        for k in range(PS_N // SUB):
            nc.tensor.matmul(
                p[:, ts(k, SUB)],
                w1t[:],
                xt[:, ds(col - base + k * SUB, SUB)],
                start=True,
                stop=True,
            )
        # strictly alternate ACT/DVE so both engines drain in parallel
        if j % 2 == 0:
            nc.scalar.activation(h[:, ts(j, PS_N)], p[:], relu)
        else:
            nc.vector.tensor_scalar_max(h[:, ts(j, PS_N)], p[:], 0.0)


def _stage2(nc, q, h, ots, w2t, ps2):
    for j in range(QUART // PS_N):
        p = ps2.tile([MP2, PS_N], F32, tag="p2", name=f"p2_{q}_{j}")
        for k in range(PS_N // SUB):
            nc.tensor.matmul(
                p[:, ts(k, SUB)],
                w2t[:],
                h[:, ds(j * PS_N + k * SUB, SUB)],
                start=True,
                stop=True,
            )
        ot = ots[j // (DMA_N // PS_N)]
        jj = j % (DMA_N // PS_N)
        # opposite phase to stage 1 so ACT and DVE stay balanced overall
        if j % 2 == 1:
            nc.vector.tensor_copy(ot[:, ts(jj, PS_N)], p[:])
        else:
            nc.scalar.copy(ot[:, ts(jj, PS_N)], p[:])


def _build_nc():
    if "nc" in _NC_CACHE:
        return _NC_CACHE["nc"]

    nc = bacc.Bacc("TRN2", target_bir_lowering=False, debug=False, num_devices=NCORES)
    et = nc.dram_tensor("et", [KP1, NT], F8E3, kind="ExternalInput").ap()
    w1d = nc.dram_tensor("w1d", [KP1, MP1], F16, kind="ExternalInput").ap()
    w2d = nc.dram_tensor("w2d", [KP2, MP2], F16, kind="ExternalInput").ap()
    o = nc.dram_tensor("o", [MP2, NT], F16, kind="ExternalOutput").ap()

    NQ = NT // QUART
    NTILE = QUART // DMA_N

    with tile.TileContext(nc) as tc:
        with (
            tc.tile_pool(name="consts", bufs=1) as consts,
            tc.tile_pool(name="inp", bufs=2 * NTILE) as inp,
            tc.tile_pool(name="hbuf", bufs=2) as hbuf,
            tc.tile_pool(name="obuf", bufs=2 * NTILE) as obuf,
            tc.tile_pool(name="ps1", bufs=2, space="PSUM") as ps1,
            tc.tile_pool(name="ps2", bufs=2, space="PSUM") as ps2,
        ):
            w1t = consts.tile([KP1, MP1], F16)
            nc.gpsimd.dma_start(w1t[:], w1d[:])
            w2t = consts.tile([KP2, MP2], F16)
            nc.gpsimd.dma_start(w2t[:], w2d[:])

            def load_tiles(q, sizes):
                # one SWDGE call per tile; bigger calls span more ~70KB
                # packets -> more SDMA engines in parallel
                xts = []
                base = 0
                for t, ncols in enumerate(sizes):
                    x = inp.tile(
                        [KP1, ncols], F8E3, tag=f"x{ncols}",
                        name=f"x_{q}_{t}", bufs=2 if ncols == QUART else 2,
                    )
                    c0 = q * QUART + base
                    # SP-queue HWDGE (walrus flag spreads it across 11
                    # engines); Pool/SWDGE stays output-only so stores can
                    # never block input prefetch and the SWDGE packet->engine
                    # spread stays even
                    nc.sync.dma_start(x[:], et[:, c0 : c0 + ncols])
                    xts.append((x, base, ncols))
                    base += ncols
                assert base == QUART
                return xts

            # progressive first quarter: small leading tiles so the first
            # matmul starts as early as possible
            xts = load_tiles(0, [1024, 1024, 2048, 4096])
            for q in range(NQ):
                h = hbuf.tile([KP2, QUART], F16, tag="h", name=f"h_{q}")
                _stage1(nc, q, xts, h, w1t, ps1)
                # prefetch next quarter's input BEFORE this quarter's output
                # stores hit the Pool queue (stores wait on casts; input must
                # not sit behind them)
                if q + 1 < NQ:
                    xts = load_tiles(q + 1, [DMA_N, DMA_N])
                ots = [
                    obuf.tile([MP2, DMA_N], F16, tag="ot", name=f"ot_{q}_{t}")
                    for t in range(NTILE)
                ]
                _stage2(nc, q, h, ots, w2t, ps2)
                for t in range(NTILE):
                    nc.gpsimd.dma_start(
                        o[:, q * QUART + t * DMA_N : q * QUART + (t + 1) * DMA_N],
                        ots[t][:],
                    )

    nc.compile()
    _NC_CACHE["nc"] = nc
    return nc


def kernel(**inputs):
    global LAST_RESULTS
    src = np.asarray(inputs["src"])
    emb = np.asarray(inputs["emb"], np.float32)
    Wp = np.asarray(inputs["Wp"], np.float32)
    bp = np.asarray(inputs["bp"], np.float32)
    C, cb = _build_C(
        inputs["w1"], inputs["b1"], inputs["w2"], inputs["b2"],
        inputs["w3"], inputs["b3"], inputs["w4"], inputs["b4"],
    )

    # stage-1 stationary [101, 101]
    L1 = np.zeros((KP1, MP1), np.float16)
    L1[0:LIVE, 0:F] = C.T
    L1[LIVE : 2 * LIVE, F : 2 * F] = C.T
    L1[2 * LIVE, 0:F] = cb
    L1[2 * LIVE, F : 2 * F] = cb
    L1[2 * LIVE, 2 * F] = 1.0  # forward the ones-row (relu(1) == 1)

    # stage-2 stationary [101, 100]
    L2 = np.zeros((KP2, MP2), np.float16)
    L2[0:F, 0:F] = Wp.T
    L2[F : 2 * F, F : 2 * F] = Wp.T
    L2[2 * F, 0:F] = bp
    L2[2 * F, F : 2 * F] = bp

    # host gather + per-core transposed layout [101, 32768]
    e = emb[src[:, : (LIVE + 2) // 3]].reshape(B, -1)[:, :LIVE]  # [B, 50]
    in_maps = []
    for c in range(NCORES):
        blk = e[c * RPC : (c + 1) * RPC].reshape(2, HALF, LIVE)
        ET = np.empty((KP1, NT), ml_dtypes.float8_e3m4)
        ET[0 : 2 * LIVE] = np.transpose(blk, (0, 2, 1)).reshape(2 * LIVE, HALF)
        ET[2 * LIVE] = 1.0
        in_maps.append({"et": ET, "w1d": L1, "w2d": L2})

    nc = _build_nc()
    trace = bool(int(os.environ.get("KERNEL_TRACE", "0")))
    res = run_bass_kernel_spmd(
        nc, in_maps, core_ids=list(range(NCORES)), trace=trace
    )
    LAST_RESULTS = res

    out = np.empty((B, F), np.float32)
    for c in range(NCORES):
        oc = res.results[c]["o"].astype(np.float32)
        out[c * RPC : c * RPC + HALF] = oc[0:F].T
        out[c * RPC + HALF : (c + 1) * RPC] = oc[F : 2 * F].T
    return out


# revision 9
# speedup vs baseline: 1.1752x; 1.1752x over previous
"""Trainium2 kernel for nn_CNNEncoder: embed(1000,3) -> 4x conv1d(stride3) -> relu -> 50x50 linear.

Math: the four stride-3 convs + concat are one linear map C [50, 60] over the
flattened embedding signal e = emb[src].reshape(B, 60). The conv windows never
touch signal positions 50..59 (max live index is 49), so C[:, 50:] == 0 and the
shipped signal is trimmed to 50 live positions per row:
    out = relu(e_live @ C_live.T + cb) @ Wp.T + bp

Device layout (pure data parallel over 8 cores, 65536 rows/core):
  - features on partitions, rows on the free dim (PE contracts over partitions)
  - two 32768-row chunks packed block-diagonally: stage-1 lhsT is [101, 101]
    (50 live signal partitions per chunk + shared ones-row for the bias; col
    100 forwards the ones-row so stage 2 gets its bias row for free), stage-2
    lhsT is [101, 100].
  - column quarters of 8192: all 16 stage-1 matmuls (one stationary), relu
    (split ACT/DVE) into an SBUF h buffer, then all 16 stage-2 matmuls (one
    stationary), PSUM->SBUF fp16 cast (split DVE/ACT). Grouping the
    stationaries lets the PE's background weight buffer hide LDWEIGHTS;
    1024-col drains amortize per-op engine overhead.
  - input signal shipped as fp8 e3m4 (1 byte; ~1.2% rel err, well under the
    2e-2 gate) and consumed directly by the PE: TRN2 matmul supports a mixed
    fp16-stationary x fp8e3-moving matmul exactly (hw-verified).
  - ALL bulk DMA via SWDGE (gpsimd.dma_start): software descriptors spray
    evenly across all 16 SDMA engines (HWDGE concentrates on few engines).
    Issue order interleaves input prefetch BEFORE output stores so a store
    waiting on casts can never head-of-line block the next quarter's input.

Host side does only data movement: the embedding gather (index lookup, no
arithmetic) and transposes for the on-device layout. All FLOPs run on device.
"""

import os
import numpy as np
import ml_dtypes

try:
    import concourse.bass as bass
except ImportError:  # grading env may not have concourse on sys.path
    import sys

    sys.path.insert(0, "/opt/trn_rl_repo")
    import concourse.bass as bass

import concourse.mybir as mybir
import concourse.tile as tile
from concourse import bacc
from concourse.bass import ds, ts
from concourse import bass_utils
from concourse.bass_utils import run_bass_kernel_spmd

# spread HWDGE DMAs across many SDMA engines (default pins them to one)
_orig_run_command = bass_utils.run_command

_FLAG = "--min-num-dma-engines-for-dge=16"
_flag_ok = None


def _walrus_supports_flag(walrus):
    global _flag_ok
    if _flag_ok is None:
        try:
            import subprocess

            out = subprocess.run(
                [walrus, "--help"], capture_output=True, text=True, timeout=120
            )
            _flag_ok = "--min-num-dma-engines-for-dge" in (out.stdout + out.stderr)
        except Exception:
            _flag_ok = False
    return _flag_ok


def _patched_run_command(argv, **kwargs):
    if (
        argv
        and "walrus_driver" in str(argv[0])
        and "--pass" in argv
        and _walrus_supports_flag(str(argv[0]))
    ):
        argv = list(argv) + [_FLAG]
    return _orig_run_command(argv, **kwargs)


bass_utils.run_command = _patched_run_command


B = 524288
SEQ = 20
EMB = 3
L = SEQ * EMB  # 60 (only 0..49 live)
LIVE = 50
F = 50
NCORES = 8
RPC = B // NCORES  # 65536 rows per core
HALF = RPC // 2  # 32768 rows per packed chunk
NT = HALF  # free dim of the per-core device tensors

KP1 = 2 * LIVE + 1  # 101: [chunkA 50 | chunkB 50 | ones]
MP1 = 2 * F + 1  # 101: [chunkA 50 | chunkB 50 | ones passthrough]
KP2 = MP1  # 101
MP2 = 2 * F  # 100

QUART = 8192
DMA_N = 4096
SUB = 512
PS_N = 1024  # psum tile cols (2 banks)

F32 = mybir.dt.float32
F16 = mybir.dt.float16
F8E3 = mybir.dt.float8e3

CONV_SPECS = [(10, 14), (12, 13), (13, 12), (15, 11)]  # (pad, n_out)

LAST_RESULTS = None  # BassKernelResults of the most recent run (for profiling)

_NC_CACHE = {}


def _build_C(w1, b1, w2, b2, w3, b3, w4, b4):
    C = np.zeros((F, L), np.float64)
    cb = np.zeros(F, np.float64)
    f = 0
    for (w, b), (pad, nout) in zip(
        [(w1, b1), (w2, b2), (w3, b3), (w4, b4)], CONV_SPECS
    ):
        wk = np.asarray(w, np.float64)[0, 0]
        K = wk.shape[0]
        for j in range(nout):
            for k in range(K):
                i = 3 * j + k - pad
                if 0 <= i < L:
                    C[f, i] += wk[k]
            cb[f] = np.asarray(b, np.float64)[0]
            f += 1
    assert np.all(C[:, LIVE:] == 0.0)
    return C[:, :LIVE].astype(np.float32), cb.astype(np.float32)


def _stage1(nc, q, xts, h, w1t, ps1):
    # xts: list of (tile, base_col_in_quarter, ncols)
    relu = mybir.ActivationFunctionType.Relu
    for j in range(QUART // PS_N):
        p = ps1.tile([MP1, PS_N], F32, tag="p1", name=f"p1_{q}_{j}")
        col = j * PS_N
        xt, base, ncols = next(
            (t, b, n) for (t, b, n) in xts if b <= col < b + n
        )
        for k in range(PS_N // SUB):
            nc.tensor.matmul(
                p[:, ts(k, SUB)],
                w1t[:],
                xt[:, ds(col - base + k * SUB, SUB)],
                start=True,
                stop=True,
            )
        # strictly alternate ACT/DVE so both engines drain in parallel
        if j % 2 == 0:
            nc.scalar.activation(h[:, ts(j, PS_N)], p[:], relu)
        else:
            nc.vector.tensor_scalar_max(h[:, ts(j, PS_N)], p[:], 0.0)


def _stage2(nc, q, h, ots, w2t, ps2):
    for j in range(QUART // PS_N):
        p = ps2.tile([MP2, PS_N], F32, tag="p2", name=f"p2_{q}_{j}")
        for k in range(PS_N // SUB):
            nc.tensor.matmul(
                p[:, ts(k, SUB)],
                w2t[:],
                h[:, ds(j * PS_N + k * SUB, SUB)],
                start=True,
                stop=True,
            )
        ot = ots[j // (DMA_N // PS_N)]
        jj = j % (DMA_N // PS_N)
        # opposite phase to stage 1 so ACT and DVE stay balanced overall
        if j % 2 == 1:
            nc.vector.tensor_copy(ot[:, ts(jj, PS_N)], p[:])
        else:
            nc.scalar.copy(ot[:, ts(jj, PS_N)], p[:])


def _build_nc():
    if "nc" in _NC_CACHE:
        return _NC_CACHE["nc"]

    nc = bacc.Bacc("TRN2", target_bir_lowering=False, debug=False, num_devices=NCORES)
    et = nc.dram_tensor("et", [KP1, NT], F8E3, kind="ExternalInput").ap()
    w1d = nc.dram_tensor("w1d", [KP1, MP1], F16, kind="ExternalInput").ap()
    w2d = nc.dram_tensor("w2d", [KP2, MP2], F16, kind="ExternalInput").ap()
    o = nc.dram_tensor("o", [MP2, NT], F16, kind="ExternalOutput").ap()

    NQ = NT // QUART
    NTILE = QUART // DMA_N

    with tile.TileContext(nc) as tc:
        with (
            tc.tile_pool(name="consts", bufs=1) as consts,
            tc.tile_pool(name="inp", bufs=2 * NTILE) as inp,
            tc.tile_pool(name="hbuf", bufs=2) as hbuf,
            tc.tile_pool(name="obuf", bufs=2 * NTILE) as obuf,
            tc.tile_pool(name="ps1", bufs=2, space="PSUM") as ps1,
            tc.tile_pool(name="ps2", bufs=2, space="PSUM") as ps2,
        ):
            w1t = consts.tile([KP1, MP1], F16)
            nc.gpsimd.dma_start(w1t[:], w1d[:])
            w2t = consts.tile([KP2, MP2], F16)
            nc.gpsimd.dma_start(w2t[:], w2d[:])

            def load_tiles(q, sizes):
                # one SWDGE call per tile; bigger calls span more ~70KB
                # packets -> more SDMA engines in parallel
                xts = []
                base = 0
                for t, ncols in enumerate(sizes):
                    x = inp.tile(
                        [KP1, ncols], F8E3, tag=f"x{ncols}",
                        name=f"x_{q}_{t}", bufs=2 if ncols == QUART else 2,
                    )
                    c0 = q * QUART + base
                    # SWDGE spreads a single uniform stream evenly across
                    # all 16 engines; the Pool queue carries ONLY weights +
                    # inputs so nothing can head-of-line block prefetch
                    nc.gpsimd.dma_start(x[:], et[:, c0 : c0 + ncols])
                    xts.append((x, base, ncols))
                    base += ncols
                assert base == QUART
                return xts

            # progressive first quarter: small leading tiles so the first
            # matmul starts as early as possible
            xts = load_tiles(0, [2048, 2048, 4096])
            for q in range(NQ):
                h = hbuf.tile([KP2, QUART], F16, tag="h", name=f"h_{q}")
                _stage1(nc, q, xts, h, w1t, ps1)
                # prefetch next quarter's input BEFORE this quarter's output
                # stores hit the Pool queue (stores wait on casts; input must
                # not sit behind them)
                if q + 1 < NQ:
                    xts = load_tiles(q + 1, [DMA_N, DMA_N])
                ots = [
                    obuf.tile([MP2, DMA_N], F16, tag="ot", name=f"ot_{q}_{t}")
                    for t in range(NTILE)
                ]
                _stage2(nc, q, h, ots, w2t, ps2)
                for t in range(NTILE):
                    # fp16 HWDGE splits each call across 11 SDMA engines; the
                    # SP queue carries only output stores, so waiting on casts
                    # never blocks anything else
                    nc.sync.dma_start(
                        o[:, q * QUART + t * DMA_N : q * QUART + (t + 1) * DMA_N],
                        ots[t][:],
                    )

    nc.compile()
    _NC_CACHE["nc"] = nc
    return nc


def kernel(**inputs):
    global LAST_RESULTS
    src = np.asarray(inputs["src"])
    emb = np.asarray(inputs["emb"], np.float32)
    Wp = np.asarray(inputs["Wp"], np.float32)
    bp = np.asarray(inputs["bp"], np.float32)
    C, cb = _build_C(
        inputs["w1"], inputs["b1"], inputs["w2"], inputs["b2"],
        inputs["w3"], inputs["b3"], inputs["w4"], inputs["b4"],
    )

    # stage-1 stationary [101, 101]
    L1 = np.zeros((KP1, MP1), np.float16)
    L1[0:LIVE, 0:F] = C.T
    L1[LIVE : 2 * LIVE, F : 2 * F] = C.T
    L1[2 * LIVE, 0:F] = cb
    L1[2 * LIVE, F : 2 * F] = cb
    L1[2 * LIVE, 2 * F] = 1.0  # forward the ones-row (relu(1) == 1)

    # stage-2 stationary [101, 100]
    L2 = np.zeros((KP2, MP2), np.float16)
    L2[0:F, 0:F] = Wp.T
    L2[F : 2 * F, F : 2 * F] = Wp.T
    L2[2 * F, 0:F] = bp
    L2[2 * F, F : 2 * F] = bp

    # host gather + per-core transposed layout [101, 32768]
    e = emb[src[:, : (LIVE + 2) // 3]].reshape(B, -1)[:, :LIVE]  # [B, 50]
    in_maps = []
    for c in range(NCORES):
        blk = e[c * RPC : (c + 1) * RPC].reshape(2, HALF, LIVE)
        ET = np.empty((KP1, NT), ml_dtypes.float8_e3m4)
        ET[0 : 2 * LIVE] = np.transpose(blk, (0, 2, 1)).reshape(2 * LIVE, HALF)
        ET[2 * LIVE] = 1.0
        in_maps.append({"et": ET, "w1d": L1, "w2d": L2})

    nc = _build_nc()
    trace = bool(int(os.environ.get("KERNEL_TRACE", "0")))
    res = run_bass_kernel_spmd(
        nc, in_maps, core_ids=list(range(NCORES)), trace=trace
    )
    LAST_RESULTS = res

    out = np.empty((B, F), np.float32)
    for c in range(NCORES):
        oc = res.results[c]["o"].astype(np.float32)
        out[c * RPC : c * RPC + HALF] = oc[0:F].T
        out[c * RPC + HALF : (c + 1) * RPC] = oc[F : 2 * F].T
    return out


# revision 10
# speedup vs baseline: 1.8063x; 1.5370x over previous
"""Trainium2 kernel for nn_CNNEncoder: embed(1000,3) -> 4x conv1d(stride3) -> relu -> 50x50 linear.

Math: the four stride-3 convs + concat are one linear map C [50, 60] over the
flattened embedding signal e = emb[src].reshape(B, 60). The conv windows never
touch signal positions 50..59 (max live index is 49), so C[:, 50:] == 0 and the
shipped signal is trimmed to 50 live positions per row:
    out = relu(e_live @ C_live.T + cb) @ Wp.T + bp

Device layout (pure data parallel over 8 cores, 65536 rows/core):
  - features on partitions, rows on the free dim (PE contracts over partitions)
  - two 32768-row chunks packed block-diagonally: stage-1 lhsT is [101, 101]
    (50 live signal partitions per chunk + shared ones-row for the bias; col
    100 forwards the ones-row so stage 2 gets its bias row for free), stage-2
    lhsT is [101, 100].
  - column quarters of 8192: all 16 stage-1 matmuls (one stationary), relu
    (split ACT/DVE) into an SBUF h buffer, then all 16 stage-2 matmuls (one
    stationary), PSUM->SBUF fp16 cast (split DVE/ACT). Grouping the
    stationaries lets the PE's background weight buffer hide LDWEIGHTS;
    1024-col drains amortize per-op engine overhead.
  - input signal shipped as fp8 e3m4 (1 byte; ~1.2% rel err, well under the
    2e-2 gate) and consumed directly by the PE: TRN2 matmul supports a mixed
    fp16-stationary x fp8e3-moving matmul exactly (hw-verified).
  - ALL bulk DMA via SWDGE (gpsimd.dma_start): software descriptors spray
    evenly across all 16 SDMA engines (HWDGE concentrates on few engines).
    Issue order interleaves input prefetch BEFORE output stores so a store
    waiting on casts can never head-of-line block the next quarter's input.

Host side does only data movement: the embedding gather (index lookup, no
arithmetic) and transposes for the on-device layout. All FLOPs run on device.
"""

import os
import numpy as np
import ml_dtypes

try:
    import concourse.bass as bass
except ImportError:  # grading env may not have concourse on sys.path
    import sys

    sys.path.insert(0, "/opt/trn_rl_repo")
    import concourse.bass as bass

import concourse.mybir as mybir
import concourse.tile as tile
from concourse import bacc
from concourse.bass import ds, ts
from concourse import bass_utils
from concourse.bass_utils import run_bass_kernel_spmd

# spread HWDGE DMAs across many SDMA engines (default pins them to one)
_orig_run_command = bass_utils.run_command

_FLAG = "--min-num-dma-engines-for-dge=16"
_flag_ok = None


def _walrus_supports_flag(walrus):
    global _flag_ok
    if _flag_ok is None:
        try:
            import subprocess

            out = subprocess.run(
                [walrus, "--help"], capture_output=True, text=True, timeout=120
            )
            _flag_ok = "--min-num-dma-engines-for-dge" in (out.stdout + out.stderr)
        except Exception:
            _flag_ok = False
    return _flag_ok


def _patched_run_command(argv, **kwargs):
    if (
        argv
        and "walrus_driver" in str(argv[0])
        and "--pass" in argv
        and _walrus_supports_flag(str(argv[0]))
    ):
        argv = list(argv) + [_FLAG]
    return _orig_run_command(argv, **kwargs)


bass_utils.run_command = _patched_run_command


B = 524288
SEQ = 20
EMB = 3
L = SEQ * EMB  # 60 (only 0..49 live)
LIVE = 50
F = 50
NCORES = 8
RPC = B // NCORES  # 65536 rows per core
HALF = RPC // 2  # 32768 rows per packed chunk
NT = HALF  # free dim of the per-core device tensors

KP1 = 2 * LIVE + 1  # 101: [chunkA 50 | chunkB 50 | ones]
MP1 = 2 * F + 1  # 101: [chunkA 50 | chunkB 50 | ones passthrough]
KP2 = MP1  # 101
MP2 = 2 * F  # 100

QUART = 8192
DMA_N = 4096
SUB = 512
PS_N = 1024  # psum tile cols (2 banks)

F32 = mybir.dt.float32
F16 = mybir.dt.float16
F8E3 = mybir.dt.float8e3

CONV_SPECS = [(10, 14), (12, 13), (13, 12), (15, 11)]  # (pad, n_out)

LAST_RESULTS = None  # BassKernelResults of the most recent run (for profiling)

_NC_CACHE = {}


def _build_C(w1, b1, w2, b2, w3, b3, w4, b4):
    C = np.zeros((F, L), np.float64)
    cb = np.zeros(F, np.float64)
    f = 0
    for (w, b), (pad, nout) in zip(
        [(w1, b1), (w2, b2), (w3, b3), (w4, b4)], CONV_SPECS
    ):
        wk = np.asarray(w, np.float64)[0, 0]
        K = wk.shape[0]
        for j in range(nout):
            for k in range(K):
                i = 3 * j + k - pad
                if 0 <= i < L:
                    C[f, i] += wk[k]
            cb[f] = np.asarray(b, np.float64)[0]
            f += 1
    assert np.all(C[:, LIVE:] == 0.0)
    return C[:, :LIVE].astype(np.float32), cb.astype(np.float32)


def _build_nc():
    if "nc" in _NC_CACHE:
        return _NC_CACHE["nc"]

    nc = bacc.Bacc("TRN2", target_bir_lowering=False, debug=False, num_devices=NCORES)
    et = nc.dram_tensor("et", [KP1, NT], F8E3, kind="ExternalInput").ap()
    w1d = nc.dram_tensor("w1d", [KP1, MP1], F16, kind="ExternalInput").ap()
    w2d = nc.dram_tensor("w2d", [KP2, MP2], F16, kind="ExternalInput").ap()
    o = nc.dram_tensor("o", [MP2, NT], F16, kind="ExternalOutput").ap()

    T = 2048  # pipeline tile (columns)
    XC = 4096  # input DMA call width (2 tiles)
    NTL = NT // T  # 16 pipeline tiles
    relu = mybir.ActivationFunctionType.Relu

    with tile.TileContext(nc) as tc:
        with (
            tc.tile_pool(name="consts", bufs=1) as consts,
            tc.tile_pool(name="inp", bufs=4) as inp,
            tc.tile_pool(name="hbuf", bufs=3) as hbuf,
            tc.tile_pool(name="obuf", bufs=6) as obuf,
            tc.tile_pool(name="ps1", bufs=2, space="PSUM") as ps1,
            tc.tile_pool(name="ps2", bufs=2, space="PSUM") as ps2,
        ):
            w1t = consts.tile([KP1, MP1], F16)
            nc.gpsimd.dma_start(w1t[:], w1d[:])
            w2t = consts.tile([KP2, MP2], F16)
            nc.gpsimd.dma_start(w2t[:], w2d[:])

            def load_x(i):
                # SWDGE input: the Pool queue carries ONLY weights+inputs, so
                # prefetch can never be blocked by stores waiting on compute
                x = inp.tile([KP1, XC], F8E3, tag="x", name=f"x_{i}")
                nc.gpsimd.dma_start(x[:], et[:, i * XC : (i + 1) * XC])
                return x

            xs = [load_x(0), load_x(1)]

            def mm1(k):
                x = xs[k // 2]
                off = (k % 2) * T
                h = hbuf.tile([KP2, T], F16, tag="h", name=f"h_{k}")
                for j in range(T // PS_N):
                    p = ps1.tile([MP1, PS_N], F32, tag="p1", name=f"p1_{k}_{j}")
                    for s in range(PS_N // SUB):
                        nc.tensor.matmul(
                            p[:, ts(s, SUB)],
                            w1t[:],
                            x[:, ds(off + j * PS_N + s * SUB, SUB)],
                            start=True,
                            stop=True,
                        )
                    # parallel drains: ACT takes one 1024 half, DVE the other
                    if j % 2 == 0:
                        nc.scalar.activation(h[:, ts(j, PS_N)], p[:], relu)
                    else:
                        nc.vector.tensor_scalar_max(h[:, ts(j, PS_N)], p[:], 0.0)
                return h

            def mm2(k, h):
                ot = obuf.tile([MP2, T], F16, tag="ot", name=f"ot_{k}")
                for j in range(T // PS_N):
                    p = ps2.tile([MP2, PS_N], F32, tag="p2", name=f"p2_{k}_{j}")
                    for s in range(PS_N // SUB):
                        nc.tensor.matmul(
                            p[:, ts(s, SUB)],
                            w2t[:],
                            h[:, ds(j * PS_N + s * SUB, SUB)],
                            start=True,
                            stop=True,
                        )
                    if j % 2 == 0:
                        nc.vector.tensor_copy(ot[:, ts(j, PS_N)], p[:])
                    else:
                        nc.scalar.copy(ot[:, ts(j, PS_N)], p[:])
                # fp16 HWDGE store: splits across 11 SDMA engines; SP queue
                # carries only stores so its cast-waits block nothing else
                nc.sync.dma_start(o[:, k * T : (k + 1) * T], ot[:])

            # 1-tile software pipeline: stage-1 of tile k runs while stage-2
            # of tile k-1 consumes the previous relu output, so the PE never
            # waits on a just-issued drain
            hs = {0: mm1(0)}
            for k in range(1, NTL):
                if k % 2 == 1 and (k + 3) // 2 < NT // XC:
                    xs.append(load_x((k + 3) // 2))
                hs[k] = mm1(k)
                mm2(k - 1, hs.pop(k - 1))
            mm2(NTL - 1, hs.pop(NTL - 1))

    nc.compile()
    _NC_CACHE["nc"] = nc
    return nc


def kernel(**inputs):
    global LAST_RESULTS
    src = np.asarray(inputs["src"])
    emb = np.asarray(inputs["emb"], np.float32)
    Wp = np.asarray(inputs["Wp"], np.float32)
    bp = np.asarray(inputs["bp"], np.float32)
    C, cb = _build_C(
        inputs["w1"], inputs["b1"], inputs["w2"], inputs["b2"],
        inputs["w3"], inputs["b3"], inputs["w4"], inputs["b4"],
    )

    # stage-1 stationary [101, 101]
    L1 = np.zeros((KP1, MP1), np.float16)
    L1[0:LIVE, 0:F] = C.T
    L1[LIVE : 2 * LIVE, F : 2 * F] = C.T
    L1[2 * LIVE, 0:F] = cb
    L1[2 * LIVE, F : 2 * F] = cb
    L1[2 * LIVE, 2 * F] = 1.0  # forward the ones-row (relu(1) == 1)

    # stage-2 stationary [101, 100]
    L2 = np.zeros((KP2, MP2), np.float16)
    L2[0:F, 0:F] = Wp.T
    L2[F : 2 * F, F : 2 * F] = Wp.T
    L2[2 * F, 0:F] = bp
    L2[2 * F, F : 2 * F] = bp

    # host gather + per-core transposed layout [101, 32768]
    e = emb[src[:, : (LIVE + 2) // 3]].reshape(B, -1)[:, :LIVE]  # [B, 50]
    in_maps = []
    for c in range(NCORES):
        blk = e[c * RPC : (c + 1) * RPC].reshape(2, HALF, LIVE)
        ET = np.empty((KP1, NT), ml_dtypes.float8_e3m4)
        ET[0 : 2 * LIVE] = np.transpose(blk, (0, 2, 1)).reshape(2 * LIVE, HALF)
        ET[2 * LIVE] = 1.0
        in_maps.append({"et": ET, "w1d": L1, "w2d": L2})

    nc = _build_nc()
    trace = bool(int(os.environ.get("KERNEL_TRACE", "0")))
    res = run_bass_kernel_spmd(
        nc, in_maps, core_ids=list(range(NCORES)), trace=trace
    )
    LAST_RESULTS = res

    out = np.empty((B, F), np.float32)
    for c in range(NCORES):
        oc = res.results[c]["o"].astype(np.float32)
        out[c * RPC : c * RPC + HALF] = oc[0:F].T
        out[c * RPC + HALF : (c + 1) * RPC] = oc[F : 2 * F].T
    return out


# revision 11
# speedup vs baseline: 1.8439x; 1.0208x over previous
"""Trainium2 kernel for nn_CNNEncoder: embed(1000,3) -> 4x conv1d(stride3) -> relu -> 50x50 linear.

Math: the four stride-3 convs + concat are one linear map C [50, 60] over the
flattened embedding signal e = emb[src].reshape(B, 60). The conv windows never
touch signal positions 50..59 (max live index is 49), so C[:, 50:] == 0 and the
shipped signal is trimmed to 50 live positions per row:
    out = relu(e_live @ C_live.T + cb) @ Wp.T + bp

Device layout (pure data parallel over 8 cores, 65536 rows/core):
  - features on partitions, rows on the free dim (PE contracts over partitions)
  - two 32768-row chunks packed block-diagonally: stage-1 lhsT is [101, 101]
    (50 live signal partitions per chunk + shared ones-row for the bias; col
    100 forwards the ones-row so stage 2 gets its bias row for free), stage-2
    lhsT is [101, 100].
  - column quarters of 8192: all 16 stage-1 matmuls (one stationary), relu
    (split ACT/DVE) into an SBUF h buffer, then all 16 stage-2 matmuls (one
    stationary), PSUM->SBUF fp16 cast (split DVE/ACT). Grouping the
    stationaries lets the PE's background weight buffer hide LDWEIGHTS;
    1024-col drains amortize per-op engine overhead.
  - input signal shipped as fp8 e3m4 (1 byte; ~1.2% rel err, well under the
    2e-2 gate) and consumed directly by the PE: TRN2 matmul supports a mixed
    fp16-stationary x fp8e3-moving matmul exactly (hw-verified).
  - ALL bulk DMA via SWDGE (gpsimd.dma_start): software descriptors spray
    evenly across all 16 SDMA engines (HWDGE concentrates on few engines).
    Issue order interleaves input prefetch BEFORE output stores so a store
    waiting on casts can never head-of-line block the next quarter's input.

Host side does only data movement: the embedding gather (index lookup, no
arithmetic) and transposes for the on-device layout. All FLOPs run on device.
"""

import os
import numpy as np
import ml_dtypes

try:
    import concourse.bass as bass
except ImportError:  # grading env may not have concourse on sys.path
    import sys

    sys.path.insert(0, "/opt/trn_rl_repo")
    import concourse.bass as bass

import concourse.mybir as mybir
import concourse.tile as tile
from concourse import bacc
from concourse.bass import ds, ts
from concourse import bass_utils
from concourse.bass_utils import run_bass_kernel_spmd

# spread HWDGE DMAs across many SDMA engines (default pins them to one)
_orig_run_command = bass_utils.run_command

_FLAG = "--min-num-dma-engines-for-dge=16"
_flag_ok = None


def _walrus_supports_flag(walrus):
    global _flag_ok
    if _flag_ok is None:
        try:
            import subprocess

            out = subprocess.run(
                [walrus, "--help"], capture_output=True, text=True, timeout=120
            )
            _flag_ok = "--min-num-dma-engines-for-dge" in (out.stdout + out.stderr)
        except Exception:
            _flag_ok = False
    return _flag_ok


def _patched_run_command(argv, **kwargs):
    if (
        argv
        and "walrus_driver" in str(argv[0])
        and "--pass" in argv
        and _walrus_supports_flag(str(argv[0]))
    ):
        argv = list(argv) + [_FLAG]
    return _orig_run_command(argv, **kwargs)


bass_utils.run_command = _patched_run_command


B = 524288
SEQ = 20
EMB = 3
L = SEQ * EMB  # 60 (only 0..49 live)
LIVE = 50
F = 50
NCORES = 8
RPC = B // NCORES  # 65536 rows per core
HALF = RPC // 2  # 32768 rows per packed chunk
NT = HALF  # free dim of the per-core device tensors

KP1 = 2 * LIVE + 1  # 101: [chunkA 50 | chunkB 50 | ones]
MP1 = 2 * F + 1  # 101: [chunkA 50 | chunkB 50 | ones passthrough]
KP2 = MP1  # 101
MP2 = 2 * F  # 100

QUART = 8192
DMA_N = 4096
SUB = 512
PS_N = 1024  # psum tile cols (2 banks)

F32 = mybir.dt.float32
F16 = mybir.dt.float16
F8E3 = mybir.dt.float8e3

CONV_SPECS = [(10, 14), (12, 13), (13, 12), (15, 11)]  # (pad, n_out)

LAST_RESULTS = None  # BassKernelResults of the most recent run (for profiling)

_NC_CACHE = {}


def _build_C(w1, b1, w2, b2, w3, b3, w4, b4):
    C = np.zeros((F, L), np.float64)
    cb = np.zeros(F, np.float64)
    f = 0
    for (w, b), (pad, nout) in zip(
        [(w1, b1), (w2, b2), (w3, b3), (w4, b4)], CONV_SPECS
    ):
        wk = np.asarray(w, np.float64)[0, 0]
        K = wk.shape[0]
        for j in range(nout):
            for k in range(K):
                i = 3 * j + k - pad
                if 0 <= i < L:
                    C[f, i] += wk[k]
            cb[f] = np.asarray(b, np.float64)[0]
            f += 1
    assert np.all(C[:, LIVE:] == 0.0)
    return C[:, :LIVE].astype(np.float32), cb.astype(np.float32)


def _build_nc():
    if "nc" in _NC_CACHE:
        return _NC_CACHE["nc"]

    nc = bacc.Bacc("TRN2", target_bir_lowering=False, debug=False, num_devices=NCORES)
    et = nc.dram_tensor("et", [KP1, NT], F8E3, kind="ExternalInput").ap()
    wbd = nc.dram_tensor("wbd", [KP1, 2048], F16, kind="ExternalInput").ap()
    o = nc.dram_tensor("o", [MP2, NT], F16, kind="ExternalOutput").ap()

    T = 2048  # pipeline tile (columns)
    XC = 4096  # input DMA call width (2 tiles)
    NTL = NT // T  # 16 pipeline tiles
    relu = mybir.ActivationFunctionType.Relu

    with tile.TileContext(nc) as tc:
        with (
            tc.tile_pool(name="consts", bufs=1) as consts,
            tc.tile_pool(name="inp", bufs=4) as inp,
            tc.tile_pool(name="hbuf", bufs=3) as hbuf,
            tc.tile_pool(name="obuf", bufs=6) as obuf,
            tc.tile_pool(name="ps1", bufs=2, space="PSUM") as ps1,
            tc.tile_pool(name="ps2", bufs=2, space="PSUM") as ps2,
        ):
            # single fat-line weight blob: 4KB/partition lines split into
            # multiple SWDGE packets -> parallel engines, ~3us instead of the
            # ~12us a 202B-per-line load takes on one engine
            wt = consts.tile([KP1, 2048], F16)
            nc.gpsimd.dma_start(wt[:], wbd[:])
            w1t = wt[:, 0:MP1]
            w2t = wt[:, ds(MP1, MP2)]

            def load_x(i):
                # SWDGE input: the Pool queue carries ONLY weights+inputs, so
                # prefetch can never be blocked by stores waiting on compute
                x = inp.tile([KP1, XC], F8E3, tag="x", name=f"x_{i}")
                nc.gpsimd.dma_start(x[:], et[:, i * XC : (i + 1) * XC])
                return x

            xs = [load_x(0), load_x(1)]

            def mm1(k):
                x = xs[k // 2]
                off = (k % 2) * T
                h = hbuf.tile([KP2, T], F16, tag="h", name=f"h_{k}")
                for j in range(T // PS_N):
                    p = ps1.tile([MP1, PS_N], F32, tag="p1", name=f"p1_{k}_{j}")
                    for s in range(PS_N // SUB):
                        nc.tensor.matmul(
                            p[:, ts(s, SUB)],
                            w1t,
                            x[:, ds(off + j * PS_N + s * SUB, SUB)],
                            start=True,
                            stop=True,
                        )
                    # parallel drains: ACT takes one 1024 half, DVE the other
                    if j % 2 == 0:
                        nc.scalar.activation(h[:, ts(j, PS_N)], p[:], relu)
                    else:
                        nc.vector.tensor_scalar_max(h[:, ts(j, PS_N)], p[:], 0.0)
                return h

            def mm2(k, h):
                ot = obuf.tile([MP2, T], F16, tag="ot", name=f"ot_{k}")
                for j in range(T // PS_N):
                    p = ps2.tile([MP2, PS_N], F32, tag="p2", name=f"p2_{k}_{j}")
                    for s in range(PS_N // SUB):
                        nc.tensor.matmul(
                            p[:, ts(s, SUB)],
                            w2t,
                            h[:, ds(j * PS_N + s * SUB, SUB)],
                            start=True,
                            stop=True,
                        )
                    if j % 2 == 0:
                        nc.vector.tensor_copy(ot[:, ts(j, PS_N)], p[:])
                    else:
                        nc.scalar.copy(ot[:, ts(j, PS_N)], p[:])
                # fp16 HWDGE store: splits across 11 SDMA engines; SP queue
                # carries only stores so its cast-waits block nothing else
                nc.sync.dma_start(o[:, k * T : (k + 1) * T], ot[:])

            # 1-tile software pipeline: stage-1 of tile k runs while stage-2
            # of tile k-1 consumes the previous relu output, so the PE never
            # waits on a just-issued drain
            hs = {0: mm1(0)}
            for k in range(1, NTL):
                if k % 2 == 1 and (k + 3) // 2 < NT // XC:
                    xs.append(load_x((k + 3) // 2))
                hs[k] = mm1(k)
                mm2(k - 1, hs.pop(k - 1))
            mm2(NTL - 1, hs.pop(NTL - 1))

    nc.compile()
    _NC_CACHE["nc"] = nc
    return nc


def kernel(**inputs):
    global LAST_RESULTS
    src = np.asarray(inputs["src"])
    emb = np.asarray(inputs["emb"], np.float32)
    Wp = np.asarray(inputs["Wp"], np.float32)
    bp = np.asarray(inputs["bp"], np.float32)
    C, cb = _build_C(
        inputs["w1"], inputs["b1"], inputs["w2"], inputs["b2"],
        inputs["w3"], inputs["b3"], inputs["w4"], inputs["b4"],
    )

    # stage-1 stationary [101, 101]
    L1 = np.zeros((KP1, MP1), np.float16)
    L1[0:LIVE, 0:F] = C.T
    L1[LIVE : 2 * LIVE, F : 2 * F] = C.T
    L1[2 * LIVE, 0:F] = cb
    L1[2 * LIVE, F : 2 * F] = cb
    L1[2 * LIVE, 2 * F] = 1.0  # forward the ones-row (relu(1) == 1)

    # stage-2 stationary [101, 100]
    L2 = np.zeros((KP2, MP2), np.float16)
    L2[0:F, 0:F] = Wp.T
    L2[F : 2 * F, F : 2 * F] = Wp.T
    L2[2 * F, 0:F] = bp
    L2[2 * F, F : 2 * F] = bp

    # host gather + per-core transposed layout [101, 32768]
    e = emb[src[:, : (LIVE + 2) // 3]].reshape(B, -1)[:, :LIVE]  # [B, 50]
    in_maps = []
    for c in range(NCORES):
        blk = e[c * RPC : (c + 1) * RPC].reshape(2, HALF, LIVE)
        ET = np.empty((KP1, NT), ml_dtypes.float8_e3m4)
        ET[0 : 2 * LIVE] = np.transpose(blk, (0, 2, 1)).reshape(2 * LIVE, HALF)
        ET[2 * LIVE] = 1.0
        WB = np.zeros((KP1, 2048), np.float16)
        WB[:, 0:MP1] = L1
        WB[:, MP1 : MP1 + MP2] = L2
        in_maps.append({"et": ET, "wbd": WB})

    nc = _build_nc()
    trace = bool(int(os.environ.get("KERNEL_TRACE", "0")))
    res = run_bass_kernel_spmd(
        nc, in_maps, core_ids=list(range(NCORES)), trace=trace
    )
    LAST_RESULTS = res

    out = np.empty((B, F), np.float32)
    for c in range(NCORES):
        oc = res.results[c]["o"].astype(np.float32)
        out[c * RPC : c * RPC + HALF] = oc[0:F].T
        out[c * RPC + HALF : (c + 1) * RPC] = oc[F : 2 * F].T
    return out


# revision 13
# speedup vs baseline: 2.5257x; 1.3697x over previous
"""Trainium2 kernel for nn_CNNEncoder: embed(1000,3) -> 4x conv1d(stride3) -> relu -> 50x50 linear.

Math: the four stride-3 convs + concat are one linear map C [50, 60] over the
flattened embedding signal e = emb[src].reshape(B, 60). So per row:
    out = relu(e @ C.T + cb) @ Wp.T + bp

Device layout (pure data parallel over 8 cores, 65536 rows/core):
  - features on partitions, rows on the free dim (PE contracts over partitions)
  - two 32768-row chunks packed block-diagonally: stage-1 lhsT is [121, 101]
    (60 signal partitions per chunk + shared ones-row for the bias; col 100
    forwards the ones-row so stage 2 gets its bias row for free), stage-2 lhsT
    is [101, 100].
  - per 512-col subtile: matmul -> relu (ACT) -> matmul -> copy to SBUF (DVE),
    DMA in/out in 4096-col super-tiles.

Host side does only data movement: the embedding gather (index lookup, no
arithmetic) and transposes for the on-device layout. All FLOPs run on device.
"""

import os
import numpy as np

try:
    import concourse.bass as bass
except ImportError:  # grading env may not have concourse on sys.path
    import sys

    sys.path.insert(0, "/opt/trn_rl_repo")
    import concourse.bass as bass

import concourse.mybir as mybir
import concourse.tile as tile
from concourse import bacc
from concourse.bass import ds, ts
from concourse import bass_utils
from concourse.bass_utils import run_bass_kernel_spmd

# spread HWDGE DMAs across all 16 SDMA engines (default leaves some idle)
_orig_run_command = bass_utils.run_command


_FLAG = "--min-num-dma-engines-for-dge=16"
_flag_ok = None


def _walrus_supports_flag(walrus):
    global _flag_ok
    if _flag_ok is None:
        try:
            import subprocess

            out = subprocess.run(
                [walrus, "--help"], capture_output=True, text=True, timeout=120
            )
            _flag_ok = "--min-num-dma-engines-for-dge" in (out.stdout + out.stderr)
        except Exception:
            _flag_ok = False
    return _flag_ok


def _patched_run_command(argv, **kwargs):
    if (
        argv
        and "walrus_driver" in str(argv[0])
        and "--pass" in argv
        and _walrus_supports_flag(str(argv[0]))
    ):
        argv = list(argv) + [_FLAG]
    return _orig_run_command(argv, **kwargs)


bass_utils.run_command = _patched_run_command

B = 524288
SEQ = 20
EMB = 3
L = SEQ * EMB  # 60
F = 50
NCORES = 8
RPC = B // NCORES  # 65536 rows per core
HALF = RPC // 2  # 32768 rows per packed chunk
NT = HALF  # free dim of the per-core device tensors

KP1 = 2 * L + 1  # 121: [chunkA 60 | chunkB 60 | ones]
MP1 = 2 * F + 1  # 101: [chunkA 50 | chunkB 50 | ones passthrough]
KP2 = MP1  # 101
MP2 = 2 * F  # 100

DMA_N = 4096
SUB = 512

F32 = mybir.dt.float32
F16 = mybir.dt.float16

CONV_SPECS = [(10, 14), (12, 13), (13, 12), (15, 11)]  # (pad, n_out)

LAST_RESULTS = None  # BassKernelResults of the most recent run (for profiling)

_NC_CACHE = {}


def _build_C(w1, b1, w2, b2, w3, b3, w4, b4):
    C = np.zeros((F, L), np.float64)
    cb = np.zeros(F, np.float64)
    f = 0
    for (w, b), (pad, nout) in zip(
        [(w1, b1), (w2, b2), (w3, b3), (w4, b4)], CONV_SPECS
    ):
        wk = np.asarray(w, np.float64)[0, 0]
        K = wk.shape[0]
        for j in range(nout):
            for k in range(K):
                i = 3 * j + k - pad
                if 0 <= i < L:
                    C[f, i] += wk[k]
            cb[f] = np.asarray(b, np.float64)[0]
            f += 1
    return C.astype(np.float32), cb.astype(np.float32)


def _build_nc():
    if "nc" in _NC_CACHE:
        return _NC_CACHE["nc"]

    nc = bacc.Bacc("TRN2", target_bir_lowering=False, debug=False, num_devices=NCORES)
    et = nc.dram_tensor("et", [KP1, NT], F16, kind="ExternalInput").ap()
    w1d = nc.dram_tensor("w1d", [KP1, MP1], F16, kind="ExternalInput").ap()
    w2d = nc.dram_tensor("w2d", [KP2, MP2], F16, kind="ExternalInput").ap()
    o = nc.dram_tensor("o", [MP2, NT], F16, kind="ExternalOutput").ap()

    # column schedule: small tiles at the edges for fast ramp/drain, big
    # DMAs in steady state to amortize descriptor generation
    col_tiles = [1024] * 2 + [2048] * 15
    assert sum(col_tiles) == NT

    with tile.TileContext(nc) as tc:
        with (
            tc.tile_pool(name="consts", bufs=1) as consts,
            tc.tile_pool(name="inp", bufs=6) as inp,
            tc.tile_pool(name="hbuf", bufs=6) as hbuf,
            tc.tile_pool(name="obuf", bufs=6) as obuf,
            tc.tile_pool(name="ps1", bufs=3, space="PSUM") as ps1,
            tc.tile_pool(name="ps2", bufs=3, space="PSUM") as ps2,
        ):
            w1t = consts.tile([KP1, MP1], F16)
            nc.sync.dma_start(w1t[:], w1d[:])
            w2t = consts.tile([KP2, MP2], F16)
            nc.sync.dma_start(w2t[:], w2d[:])

            col = 0
            for i, ncols in enumerate(col_tiles):
                x = inp.tile([KP1, ncols], F16, tag="x")
                nc.sync.dma_start(x[:], et[:, col : col + ncols])
                ot = obuf.tile([MP2, ncols], F16, tag="ot")
                for j2 in range(ncols // (2 * SUB)):
                    # pair subtiles so each stationary is loaded once per two
                    # matmuls (PE pulls the next LDWEIGHTS ahead while the
                    # current matmul streams) and the two PSUM drains run on
                    # ACT and DVE in parallel
                    p1a = ps1.tile([MP1, SUB], F32, tag="p1", name=f"p1a_{i}_{j2}")
                    nc.tensor.matmul(
                        p1a[:], w1t[:], x[:, ts(2 * j2, SUB)], start=True, stop=True
                    )
                    p1b = ps1.tile([MP1, SUB], F32, tag="p1", name=f"p1b_{i}_{j2}")
                    nc.tensor.matmul(
                        p1b[:], w1t[:], x[:, ts(2 * j2 + 1, SUB)], start=True, stop=True
                    )
                    ha = hbuf.tile([KP2, SUB], F16, tag="h", name=f"ha_{i}_{j2}")
                    nc.scalar.activation(
                        ha[:], p1a[:], mybir.ActivationFunctionType.Relu
                    )
                    hb = hbuf.tile([KP2, SUB], F16, tag="h", name=f"hb_{i}_{j2}")
                    nc.vector.tensor_scalar_max(hb[:], p1b[:], 0.0)
                    p2a = ps2.tile([MP2, SUB], F32, tag="p2", name=f"p2a_{i}_{j2}")
                    nc.tensor.matmul(p2a[:], w2t[:], ha[:], start=True, stop=True)
                    p2b = ps2.tile([MP2, SUB], F32, tag="p2", name=f"p2b_{i}_{j2}")
                    nc.tensor.matmul(p2b[:], w2t[:], hb[:], start=True, stop=True)
                    nc.vector.tensor_copy(ot[:, ts(2 * j2, SUB)], p2a[:])
                    nc.scalar.copy(ot[:, ts(2 * j2 + 1, SUB)], p2b[:])
                # stores via SWDGE: descriptors spray across all 16 SDMA
                # engines, while HWDGE loads are pinned to the 11 model rows
                nc.gpsimd.dma_start(o[:, col : col + ncols], ot[:])
                col += ncols

    nc.compile()
    _NC_CACHE["nc"] = nc
    return nc


def kernel(**inputs):
    global LAST_RESULTS
    src = np.asarray(inputs["src"])
    emb = np.asarray(inputs["emb"], np.float32)
    Wp = np.asarray(inputs["Wp"], np.float32)
    bp = np.asarray(inputs["bp"], np.float32)
    C, cb = _build_C(
        inputs["w1"], inputs["b1"], inputs["w2"], inputs["b2"],
        inputs["w3"], inputs["b3"], inputs["w4"], inputs["b4"],
    )

    # stage-1 stationary [121, 101]
    L1 = np.zeros((KP1, MP1), np.float16)
    L1[0:L, 0:F] = C.T
    L1[L : 2 * L, F : 2 * F] = C.T
    L1[2 * L, 0:F] = cb
    L1[2 * L, F : 2 * F] = cb
    L1[2 * L, 2 * F] = 1.0  # forward the ones-row (relu(1) == 1)

    # stage-2 stationary [101, 100]
    L2 = np.zeros((KP2, MP2), np.float16)
    L2[0:F, 0:F] = Wp.T
    L2[F : 2 * F, F : 2 * F] = Wp.T
    L2[2 * F, 0:F] = bp
    L2[2 * F, F : 2 * F] = bp

    # host gather + per-core transposed layout [121, 32768]
    e = emb[src]  # [B, 20, 3]
    in_maps = []
    for c in range(NCORES):
        blk = e[c * RPC : (c + 1) * RPC].reshape(2, HALF, L)
        ET = np.empty((KP1, NT), np.float16)
        ET[0 : 2 * L] = np.transpose(blk, (0, 2, 1)).reshape(2 * L, HALF)
        ET[2 * L] = 1.0
        in_maps.append({"et": ET, "w1d": L1, "w2d": L2})

    nc = _build_nc()
    trace = bool(int(os.environ.get("KERNEL_TRACE", "0")))
    res = run_bass_kernel_spmd(
        nc, in_maps, core_ids=list(range(NCORES)), trace=trace
    )
    LAST_RESULTS = res

    out = np.empty((B, F), np.float32)
    for c in range(NCORES):
        oc = res.results[c]["o"].astype(np.float32)
        out[c * RPC : c * RPC + HALF] = oc[0:F].T
        out[c * RPC + HALF : (c + 1) * RPC] = oc[F : 2 * F].T
    return out

